# revision 10
# baseline (speedup 1.0000x reference)
"""CrossInvolution kernel for 8 Trainium2 NeuronCores.

Math (per batch b):
  t      = relu(bn(w1 @ guide))                       # [RED=64, H*W]
  weight = w2 @ t + b2                                # [G*K*K=784, H*W]
  out[c,p] = sum_k weight[g(c)*49+k, p] * x[c, p+dk] + x[c, p]

Sharding: 8 cores = 2 batches x 4 row-slices of 16 image rows each.
Each core computes its full pipeline on its slice (halo rows come in
via host-side padding); no cross-core communication.

Engine plan (v2):
  - PE: mm1, then one fused "broadcast" matmul per tap with
    lhsT = w2e[65,128] (w2 rearranged per channel; row 64 carries the
    b2 bias via a ones-row appended to t; center tap bias += 1 folds
    in the +x residual). PSUM directly holds per-channel tap weights.
  - Scalar (ACT): BN+relu on t, PSUM->SBUF fp16 weight copies.
  - DVE: the involution multiplies and adds only, all contiguous fp16
    SBUF ops (2x mode). x arrives as 7 kw-shifted contiguous copies
    DMA'd straight from HBM (host ships the padded fp16 feature map).
  - GPSIMD: unused (concurrent GPSIMD SBUF reads starve the DVE).
"""

import numpy as np

import concourse.bass as bass
import concourse.bacc as bacc
import concourse.mybir as mybir
import concourse.tile as tile
from concourse.bass_utils import run_bass_kernel_spmd

FP = mybir.dt.float32
HP = mybir.dt.float16
N_CORES = 8
C = 256
RED = 64
G = 16
GC = 16
KS = 7
KK = KS * KS  # 49
H = W = 64
ROWS = 16          # image rows per core
PIX = ROWS * W     # 1024 pixels per core
PROWS = ROWS + 6   # padded rows (halo 3 each side)
PW = W + 6         # padded width
XROW = PROWS * W   # 1408: one kw-shifted copy, rows contiguous at 64

TRACE = False
LAST_RESULTS = None

_CACHED_NC = None


def _build_nc():
    nc = bacc.Bacc(
        "TRN2",
        debug=False,
        target_bir_lowering=False,
        num_devices=N_CORES,
    )

    guide = nc.dram_tensor("guide", (C, ROWS, W), HP, kind="ExternalInput")
    # feat7[c, kw, r, x] = padded_feature[c, r, x + kw] (host-shifted, so
    # every load and every DVE multiply is contiguous at 64-wide rows)
    feat7 = nc.dram_tensor("feat7", (C, KS, PROWS, W), HP,
                           kind="ExternalInput")
    w1t = nc.dram_tensor("w1t", (C, RED), HP, kind="ExternalInput")
    # w2a[r, ((i*7+kw)*7+kh)*128 + c] = w2[(c//16+8i)*49 + kh*7+kw, r];
    # row RED carries b2 (+1 on center tap -> +x residual)
    w2a = nc.dram_tensor("w2a", (RED + 1, 2 * KK * 128), HP,
                         kind="ExternalInput")
    scl = nc.dram_tensor("scl", (RED, 1), FP, kind="ExternalInput")
    shf = nc.dram_tensor("shf", (RED, 1), FP, kind="ExternalInput")
    out = nc.dram_tensor("out", (C, ROWS, W), HP, kind="ExternalOutput")

    MUL = mybir.AluOpType.mult
    ADD = mybir.AluOpType.add
    # taps per PSUM chunk: {2,2,2,1} per kw-group (PSUM = 8 banks =
    # 4096 fp32; 2-tap chunk = 2048 fp32 = 4 banks, ping-ponged)
    CHUNKS = [(0, 2), (2, 2), (4, 2), (6, 1)]

    with tile.TileContext(nc) as tc:
        with (
            tc.tile_pool(name="consts", bufs=1) as consts,
            tc.tile_pool(name="big", bufs=1) as big,
            tc.tile_pool(name="work", bufs=3) as work,
            tc.tile_pool(name="psum", bufs=2, space="PSUM") as psum,
        ):
            # ---- ACT table preload: tiny dep-free op at t=0 ----
            warm = consts.tile([1, 1], FP)
            nc.vector.memset(warm, 0.0)
            nc.scalar.activation(warm, warm,
                                 mybir.ActivationFunctionType.Relu)

            # ---- loads ----
            scl_sb = consts.tile([RED, 1], FP)
            nc.sync.dma_start(scl_sb, scl.ap())
            shf_sb = consts.tile([RED, 1], FP)
            nc.sync.dma_start(shf_sb, shf.ap())
            w1t_sb = [consts.tile([128, RED], HP, tag=f"w1t{i}",
                                  name=f"w1t{i}") for i in range(2)]
            guide_sb = [big.tile([128, PIX], HP, tag=f"gd{i}", name=f"gd{i}")
                        for i in range(2)]
            for i in range(2):
                csl = slice(i * 128, (i + 1) * 128)
                nc.sync.dma_start(
                    guide_sb[i],
                    guide.ap()[csl].rearrange("p a b -> p (a b)"))
                nc.sync.dma_start(w1t_sb[i], w1t.ap()[i * 128:(i + 1) * 128])
            w2a_sb = consts.tile([RED + 1, 2 * KK * 128], HP)
            # per-(half, kw) chunks of w2a and xk stream in just-in-time
            # (emitted inside the kw loop, after the first groups' chunks)
            xk = [big.tile([128, KS * XROW], HP, tag=f"xk{i}", name=f"xk{i}")
                  for i in range(2)]

            def load_group(i, kw):
                csl = slice(i * 128, (i + 1) * 128)
                nc.sync.dma_start(
                    w2a_sb[:, (i * KS + kw) * KS * 128:
                           (i * KS + kw + 1) * KS * 128],
                    w2a.ap()[:, (i * KS + kw) * KS * 128:
                             (i * KS + kw + 1) * KS * 128])
                nc.sync.dma_start(
                    xk[i][:, kw * XROW:(kw + 1) * XROW],
                    feat7.ap()[csl, kw].rearrange("p a b -> p (a b)"))

            # first two groups' weight/feature chunks go on the queue now
            load_group(0, 0)
            load_group(0, 1)

            # ---- mm1 + BN/relu -> t (fp16), plus ones row for bias ----
            t_sb = big.tile([RED + 1, PIX], HP)
            nc.vector.memset(t_sb[RED:RED + 1, :], 1.0)
            for nh in range(2):
                nsl = slice(nh * 512, (nh + 1) * 512)
                t_ps = psum.tile([RED, 512], FP, tag="wb",
                                 padded_shape=[RED, 2048])
                for i in range(2):
                    nc.tensor.matmul(
                        t_ps,
                        w1t_sb[i],
                        guide_sb[i][:, nsl],
                        start=(i == 0),
                        stop=(i == 1),
                    )
                nc.scalar.activation(
                    t_sb[0:RED, nsl], t_ps,
                    mybir.ActivationFunctionType.Relu,
                    bias=shf_sb[:, :], scale=scl_sb[:, :],
                )

            # ---- involution ----
            for i in range(2):
                run = big.tile([128, PIX], HP, tag=f"run{i}", name=f"run{i}")
                osb = big.tile([128, PIX], HP, tag=f"osb{i}", name=f"osb{i}")
                for kw in range(KS):
                    # prefetch the group two steps ahead
                    nxt = i * KS + kw + 2
                    if nxt < 2 * KS:
                        load_group(nxt // KS, nxt % KS)
                    # broadcast matmuls + fp16 copy for the 7 kh-taps
                    wsb = work.tile([128, KS * PIX], HP, tag="wsb", bufs=3)
                    prod = work.tile([128, KS * PIX], HP, tag="prod", bufs=3)
                    head = (i == 0 and kw == 0)
                    for (k0, nk) in CHUNKS:
                        w_ps = psum.tile([128, nk * PIX], FP, tag="wb",
                                         padded_shape=[128, 2048])
                        for u in range(nk):
                            kh = k0 + u
                            lhs = w2a_sb[:, ((i * KS + kw) * KS + kh) * 128:
                                         ((i * KS + kw) * KS + kh + 1) * 128]
                            for nh in range(2):
                                nc.tensor.matmul(
                                    w_ps[:, u * PIX + nh * 512:
                                         u * PIX + nh * 512 + 512],
                                    lhs,
                                    t_sb[:, nh * 512:nh * 512 + 512],
                                    start=True,
                                    stop=True,
                                )
                        nc.scalar.activation(
                            wsb[:, k0 * PIX:(k0 + nk) * PIX], w_ps,
                            mybir.ActivationFunctionType.Copy,
                        )
                        if head:
                            # pipeline head: multiply per-chunk so the
                            # DVE starts before the whole group is copied
                            xs = bass.AP(
                                tensor=xk[i].tensor,
                                offset=xk[i].offset + kw * XROW + k0 * W,
                                ap=[xk[i].ap[0], [W, nk], [1, PIX]],
                            )
                            nc.vector.tensor_tensor(
                                prod[:, k0 * PIX:(k0 + nk) * PIX].rearrange(
                                    "p (a b) -> p a b", b=PIX),
                                xs,
                                wsb[:, k0 * PIX:(k0 + nk) * PIX].rearrange(
                                    "p (a b) -> p a b", b=PIX),
                                MUL)
                    if not head:
                        # multiply all 7 kh-taps: runs of 1024 contiguous
                        xs = bass.AP(
                            tensor=xk[i].tensor,
                            offset=xk[i].offset + kw * XROW,
                            ap=[xk[i].ap[0], [W, KS], [1, PIX]],
                        )
                        nc.vector.tensor_tensor(
                            prod.rearrange("p (a b) -> p a b", b=PIX),
                            xs,
                            wsb.rearrange("p (a b) -> p a b", b=PIX),
                            MUL)
                    # 7 -> 1 tree (contiguous fp16), accumulated into run
                    s2 = work.tile([128, 3 * PIX], HP, tag="s2", bufs=3)
                    nc.vector.tensor_tensor(
                        s2, prod[:, 0:3 * PIX], prod[:, 3 * PIX:6 * PIX], ADD)
                    ab = work.tile([128, 2 * PIX], HP, tag="ab", bufs=3)
                    nc.vector.tensor_tensor(
                        ab[:, 0:PIX], s2[:, 0:PIX], s2[:, PIX:2 * PIX], ADD)
                    nc.vector.tensor_tensor(
                        ab[:, PIX:2 * PIX], s2[:, 2 * PIX:3 * PIX],
                        prod[:, 6 * PIX:7 * PIX], ADD)
                    if kw == 0:
                        nc.vector.tensor_tensor(
                            run, ab[:, 0:PIX], ab[:, PIX:2 * PIX], ADD)
                    else:
                        g = work.tile([128, PIX], HP, tag="g", bufs=3)
                        nc.vector.tensor_tensor(
                            g, ab[:, 0:PIX], ab[:, PIX:2 * PIX], ADD)
                        dst = osb if kw == KS - 1 else run
                        nc.vector.tensor_tensor(dst, run, g, ADD)
                nc.sync.dma_start(
                    out.ap()[i * 128:(i + 1) * 128].rearrange(
                        "p a b -> p (a b)"),
                    osb)

    nc.compile()
    return nc


def kernel(**inputs):
    global _CACHED_NC, LAST_RESULTS
    feature_map = np.asarray(inputs["feature_map"], np.float32)
    guide_map = np.asarray(inputs["guide_map"], np.float32)
    w1 = np.asarray(inputs["w1"], np.float32)
    bn_gamma = np.asarray(inputs["bn_gamma"], np.float32)
    bn_beta = np.asarray(inputs["bn_beta"], np.float32)
    bn_mean = np.asarray(inputs["bn_mean"], np.float32)
    bn_var = np.asarray(inputs["bn_var"], np.float32)
    w2 = np.asarray(inputs["w2"], np.float32)
    b2 = np.asarray(inputs["b2"], np.float32)

    scale = bn_gamma / np.sqrt(bn_var + 1e-5)
    shift = bn_beta - bn_mean * scale
    w1t = np.ascontiguousarray(w1.T).astype(np.float16)    # [256, 64]

    # w2a[r, i, kw, kh, c] = w2[(c//16 + 8i)*49 + kh*7+kw, r]; row RED =
    # b2 bias (+1 on the center tap: folds the +x residual in)
    w2g = w2.reshape(G, KS, KS, RED)                        # [g, kh, kw, r]
    b2g = b2.reshape(G, KS, KS).copy()                      # [g, kh, kw]
    b2g[:, 3, 3] += 1.0
    w2a = np.zeros((RED + 1, 2, KS, KS, 128), np.float32)
    for i in range(2):
        gidx = np.arange(128) // GC + 8 * i                 # [c] -> g
        # [r, kw, kh, c]
        w2a[0:RED, i] = w2g[gidx].transpose(3, 2, 1, 0)
        w2a[RED, i] = b2g[gidx].transpose(2, 1, 0)
    w2a = np.ascontiguousarray(
        w2a.reshape(RED + 1, 2 * KK * 128)).astype(np.float16)

    fpad = np.pad(feature_map, ((0, 0), (0, 0), (3, 3), (3, 3))).astype(
        np.float16)
    # feat7[b, c, kw, r, x] = fpad[b, c, r, x + kw]
    feat7 = np.stack([fpad[:, :, :, kw:kw + W] for kw in range(KS)], axis=2)

    in_maps = []
    for core in range(N_CORES):
        b, q = divmod(core, 4)
        r0 = q * ROWS
        in_maps.append(dict(
            guide=np.ascontiguousarray(
                guide_map[b, :, r0:r0 + ROWS, :]).astype(np.float16),
            feat7=np.ascontiguousarray(feat7[b, :, :, r0:r0 + PROWS, :]),
            w1t=w1t, w2a=w2a,
            scl=scale.reshape(-1, 1), shf=shift.reshape(-1, 1),
        ))

    if _CACHED_NC is None:
        _CACHED_NC = _build_nc()
    nc = _CACHED_NC

    br = run_bass_kernel_spmd(
        nc, in_maps, list(range(N_CORES)), trace=TRACE,
    )
    LAST_RESULTS = br

    out = np.empty((2, C, H, W), np.float32)
    for core in range(N_CORES):
        b, q = divmod(core, 4)
        r0 = q * ROWS
        out[b, :, r0:r0 + ROWS, :] = br.results[core]["out"].astype(
            np.float32)
    return out


# revision 11
# speedup vs baseline: 1.1869x; 1.1869x over previous
"""CrossInvolution kernel for 8 Trainium2 NeuronCores.

Math (per batch b):
  t      = relu(bn(w1 @ guide))                       # [RED=64, H*W]
  weight = w2 @ t + b2                                # [G*K*K=784, H*W]
  out[c,p] = sum_k weight[g(c)*49+k, p] * x[c, p+dk] + x[c, p]

Sharding: 8 cores = 2 batches x 4 row-slices of 16 image rows each.
Each core computes its full pipeline on its slice (halo rows come in
via host-side padding); no cross-core communication.

Engine plan (v2):
  - PE: mm1, then one fused "broadcast" matmul per tap with
    lhsT = w2e[65,128] (w2 rearranged per channel; row 64 carries the
    b2 bias via a ones-row appended to t; center tap bias += 1 folds
    in the +x residual). PSUM directly holds per-channel tap weights.
  - Scalar (ACT): BN+relu on t, PSUM->SBUF fp16 weight copies.
  - DVE: the involution multiplies and adds only, all contiguous fp16
    SBUF ops (2x mode). x arrives as 7 kw-shifted contiguous copies
    DMA'd straight from HBM (host ships the padded fp16 feature map).
  - GPSIMD: unused (concurrent GPSIMD SBUF reads starve the DVE).
"""

import numpy as np

import concourse.bass as bass
import concourse.bacc as bacc
import concourse.mybir as mybir
import concourse.tile as tile
from concourse.bass_utils import run_bass_kernel_spmd

FP = mybir.dt.float32
HP = mybir.dt.float16
N_CORES = 8
C = 256
RED = 64
G = 16
GC = 16
KS = 7
KK = KS * KS  # 49
H = W = 64
ROWS = 16          # image rows per core
PIX = ROWS * W     # 1024 pixels per core
PROWS = ROWS + 6   # padded rows (halo 3 each side)
PW = W + 6         # padded width
XROW = PROWS * W   # 1408: one kw-shifted copy, rows contiguous at 64

TRACE = False
LAST_RESULTS = None

_CACHED_NC = None


def _build_nc():
    nc = bacc.Bacc(
        "TRN2",
        debug=False,
        target_bir_lowering=False,
        num_devices=N_CORES,
    )

    guide = nc.dram_tensor("guide", (C, ROWS, W), HP, kind="ExternalInput")
    # feat7[c, kw, r, x] = padded_feature[c, r, x + kw] (host-shifted, so
    # every load and every DVE multiply is contiguous at 64-wide rows)
    feat7 = nc.dram_tensor("feat7", (C, KS, PROWS, W), HP,
                           kind="ExternalInput")
    w1t = nc.dram_tensor("w1t", (C, RED), HP, kind="ExternalInput")
    # w2a[r, ((i*7+kw)*7+kh)*128 + c] = w2[(c//16+8i)*49 + kh*7+kw, r];
    # row RED carries b2 (+1 on center tap -> +x residual)
    w2a = nc.dram_tensor("w2a", (RED + 1, 2 * KK * 128), HP,
                         kind="ExternalInput")
    scl = nc.dram_tensor("scl", (RED, 1), FP, kind="ExternalInput")
    shf = nc.dram_tensor("shf", (RED, 1), FP, kind="ExternalInput")
    out = nc.dram_tensor("out", (C, ROWS, W), HP, kind="ExternalOutput")

    MUL = mybir.AluOpType.mult
    ADD = mybir.AluOpType.add
    # taps per PSUM chunk: {2,2,2,1} per kw-group (PSUM = 8 banks =
    # 4096 fp32; 2-tap chunk = 2048 fp32 = 4 banks, ping-ponged)
    CHUNKS = [(0, 2), (2, 2), (4, 2), (6, 1)]

    with tile.TileContext(nc) as tc:
        with (
            tc.tile_pool(name="consts", bufs=1) as consts,
            tc.tile_pool(name="big", bufs=1) as big,
            tc.tile_pool(name="work", bufs=3) as work,
            tc.tile_pool(name="psum", bufs=2, space="PSUM") as psum,
        ):
            # ---- ACT table preload: tiny dep-free op at t=0 ----
            warm = consts.tile([1, 1], FP)
            nc.vector.memset(warm, 0.0)
            nc.scalar.activation(warm, warm,
                                 mybir.ActivationFunctionType.Relu)

            # ---- loads ----
            scl_sb = consts.tile([RED, 1], FP)
            nc.sync.dma_start(scl_sb, scl.ap())
            shf_sb = consts.tile([RED, 1], FP)
            nc.sync.dma_start(shf_sb, shf.ap())
            w1t_sb = [consts.tile([128, RED], HP, tag=f"w1t{i}",
                                  name=f"w1t{i}") for i in range(2)]
            guide_sb = [big.tile([128, PIX], HP, tag=f"gd{i}", name=f"gd{i}")
                        for i in range(2)]
            for i in range(2):
                csl = slice(i * 128, (i + 1) * 128)
                nc.sync.dma_start(
                    guide_sb[i],
                    guide.ap()[csl].rearrange("p a b -> p (a b)"))
                nc.sync.dma_start(w1t_sb[i], w1t.ap()[i * 128:(i + 1) * 128])
            w2a_sb = consts.tile([RED + 1, 2 * KK * 128], HP)
            # per-(half, kw) chunks of w2a and xk stream in just-in-time
            # (emitted inside the kw loop, after the first groups' chunks)
            xk = [big.tile([128, KS * XROW], HP, tag=f"xk{i}", name=f"xk{i}")
                  for i in range(2)]

            def load_group(i, kw):
                csl = slice(i * 128, (i + 1) * 128)
                nc.sync.dma_start(
                    w2a_sb[:, (i * KS + kw) * KS * 128:
                           (i * KS + kw + 1) * KS * 128],
                    w2a.ap()[:, (i * KS + kw) * KS * 128:
                             (i * KS + kw + 1) * KS * 128])
                nc.sync.dma_start(
                    xk[i][:, kw * XROW:(kw + 1) * XROW],
                    feat7.ap()[csl, kw].rearrange("p a b -> p (a b)"))

            # first two groups' weight/feature chunks go on the queue now
            load_group(0, 0)
            load_group(0, 1)

            # ---- mm1 + BN/relu -> t (fp16), plus ones row for bias ----
            t_sb = big.tile([RED + 1, PIX], HP)
            nc.vector.memset(t_sb[RED:RED + 1, :], 1.0)
            for nh in range(2):
                nsl = slice(nh * 512, (nh + 1) * 512)
                t_ps = psum.tile([RED, 512], FP, tag="wb",
                                 padded_shape=[RED, 2048])
                for i in range(2):
                    nc.tensor.matmul(
                        t_ps,
                        w1t_sb[i],
                        guide_sb[i][:, nsl],
                        start=(i == 0),
                        stop=(i == 1),
                    )
                nc.scalar.activation(
                    t_sb[0:RED, nsl], t_ps,
                    mybir.ActivationFunctionType.Relu,
                    bias=shf_sb[:, :], scale=scl_sb[:, :],
                )

            # ---- involution ----
            for i in range(2):
                run = big.tile([128, PIX], HP, tag=f"run{i}", name=f"run{i}")
                osb = big.tile([128, PIX], HP, tag=f"osb{i}", name=f"osb{i}")
                for kw in range(KS):
                    # prefetch the group two steps ahead
                    nxt = i * KS + kw + 2
                    if nxt < 2 * KS:
                        load_group(nxt // KS, nxt % KS)
                    # broadcast matmuls + fp16 copy for the 7 kh-taps
                    wsb = work.tile([128, KS * PIX], HP, tag="wsb", bufs=2)
                    prod = work.tile([128, KS * PIX], HP, tag="prod", bufs=2)
                    head = (i == 0 and kw == 0)
                    for (k0, nk) in CHUNKS:
                        w_ps = psum.tile([128, nk * PIX], FP, tag="wb",
                                         padded_shape=[128, 2048])
                        for u in range(nk):
                            kh = k0 + u
                            lhs = w2a_sb[:, ((i * KS + kw) * KS + kh) * 128:
                                         ((i * KS + kw) * KS + kh + 1) * 128]
                            for nh in range(2):
                                nc.tensor.matmul(
                                    w_ps[:, u * PIX + nh * 512:
                                         u * PIX + nh * 512 + 512],
                                    lhs,
                                    t_sb[:, nh * 512:nh * 512 + 512],
                                    start=True,
                                    stop=True,
                                )
                        nc.scalar.activation(
                            wsb[:, k0 * PIX:(k0 + nk) * PIX], w_ps,
                            mybir.ActivationFunctionType.Copy,
                        )
                        if head:
                            # pipeline head: multiply per-chunk so the
                            # DVE starts before the whole group is copied
                            xs = bass.AP(
                                tensor=xk[i].tensor,
                                offset=xk[i].offset + kw * XROW + k0 * W,
                                ap=[xk[i].ap[0], [W, nk], [1, PIX]],
                            )
                            nc.vector.tensor_tensor(
                                prod[:, k0 * PIX:(k0 + nk) * PIX].rearrange(
                                    "p (a b) -> p a b", b=PIX),
                                xs,
                                wsb[:, k0 * PIX:(k0 + nk) * PIX].rearrange(
                                    "p (a b) -> p a b", b=PIX),
                                MUL)
                    if not head:
                        # multiply all 7 kh-taps: runs of 1024 contiguous
                        xs = bass.AP(
                            tensor=xk[i].tensor,
                            offset=xk[i].offset + kw * XROW,
                            ap=[xk[i].ap[0], [W, KS], [1, PIX]],
                        )
                        nc.vector.tensor_tensor(
                            prod.rearrange("p (a b) -> p a b", b=PIX),
                            xs,
                            wsb.rearrange("p (a b) -> p a b", b=PIX),
                            MUL)
                    # 7 -> 1 tree (contiguous fp16), accumulated into run
                    s2 = work.tile([128, 3 * PIX], HP, tag="s2", bufs=2)
                    nc.vector.tensor_tensor(
                        s2, prod[:, 0:3 * PIX], prod[:, 3 * PIX:6 * PIX], ADD)
                    ab = work.tile([128, 2 * PIX], HP, tag="ab", bufs=2)
                    nc.vector.tensor_tensor(
                        ab[:, 0:PIX], s2[:, 0:PIX], s2[:, PIX:2 * PIX], ADD)
                    nc.vector.tensor_tensor(
                        ab[:, PIX:2 * PIX], s2[:, 2 * PIX:3 * PIX],
                        prod[:, 6 * PIX:7 * PIX], ADD)
                    if kw == 0:
                        nc.vector.tensor_tensor(
                            run, ab[:, 0:PIX], ab[:, PIX:2 * PIX], ADD)
                    else:
                        g = work.tile([128, PIX], HP, tag="g", bufs=2)
                        nc.vector.tensor_tensor(
                            g, ab[:, 0:PIX], ab[:, PIX:2 * PIX], ADD)
                        dst = osb if kw == KS - 1 else run
                        nc.vector.tensor_tensor(dst, run, g, ADD)
                nc.sync.dma_start(
                    out.ap()[i * 128:(i + 1) * 128].rearrange(
                        "p a b -> p (a b)"),
                    osb)

    nc.compile()
    return nc


def kernel(**inputs):
    global _CACHED_NC, LAST_RESULTS
    feature_map = np.asarray(inputs["feature_map"], np.float32)
    guide_map = np.asarray(inputs["guide_map"], np.float32)
    w1 = np.asarray(inputs["w1"], np.float32)
    bn_gamma = np.asarray(inputs["bn_gamma"], np.float32)
    bn_beta = np.asarray(inputs["bn_beta"], np.float32)
    bn_mean = np.asarray(inputs["bn_mean"], np.float32)
    bn_var = np.asarray(inputs["bn_var"], np.float32)
    w2 = np.asarray(inputs["w2"], np.float32)
    b2 = np.asarray(inputs["b2"], np.float32)

    scale = bn_gamma / np.sqrt(bn_var + 1e-5)
    shift = bn_beta - bn_mean * scale
    w1t = np.ascontiguousarray(w1.T).astype(np.float16)    # [256, 64]

    # w2a[r, i, kw, kh, c] = w2[(c//16 + 8i)*49 + kh*7+kw, r]; row RED =
    # b2 bias (+1 on the center tap: folds the +x residual in)
    w2g = w2.reshape(G, KS, KS, RED)                        # [g, kh, kw, r]
    b2g = b2.reshape(G, KS, KS).copy()                      # [g, kh, kw]
    b2g[:, 3, 3] += 1.0
    w2a = np.zeros((RED + 1, 2, KS, KS, 128), np.float32)
    for i in range(2):
        gidx = np.arange(128) // GC + 8 * i                 # [c] -> g
        # [r, kw, kh, c]
        w2a[0:RED, i] = w2g[gidx].transpose(3, 2, 1, 0)
        w2a[RED, i] = b2g[gidx].transpose(2, 1, 0)
    w2a = np.ascontiguousarray(
        w2a.reshape(RED + 1, 2 * KK * 128)).astype(np.float16)

    fpad = np.pad(feature_map, ((0, 0), (0, 0), (3, 3), (3, 3))).astype(
        np.float16)
    # feat7[b, c, kw, r, x] = fpad[b, c, r, x + kw]
    feat7 = np.stack([fpad[:, :, :, kw:kw + W] for kw in range(KS)], axis=2)

    in_maps = []
    for core in range(N_CORES):
        b, q = divmod(core, 4)
        r0 = q * ROWS
        in_maps.append(dict(
            guide=np.ascontiguousarray(
                guide_map[b, :, r0:r0 + ROWS, :]).astype(np.float16),
            feat7=np.ascontiguousarray(feat7[b, :, :, r0:r0 + PROWS, :]),
            w1t=w1t, w2a=w2a,
            scl=scale.reshape(-1, 1), shf=shift.reshape(-1, 1),
        ))

    if _CACHED_NC is None:
        _CACHED_NC = _build_nc()
    nc = _CACHED_NC

    br = run_bass_kernel_spmd(
        nc, in_maps, list(range(N_CORES)), trace=TRACE,
    )
    LAST_RESULTS = br

    out = np.empty((2, C, H, W), np.float32)
    for core in range(N_CORES):
        b, q = divmod(core, 4)
        r0 = q * ROWS
        out[b, :, r0:r0 + ROWS, :] = br.results[core]["out"].astype(
            np.float32)
    return out


# revision 12
# speedup vs baseline: 1.1895x; 1.0022x over previous
"""CrossInvolution kernel for 8 Trainium2 NeuronCores.

Math (per batch b):
  t      = relu(bn(w1 @ guide))                       # [RED=64, H*W]
  weight = w2 @ t + b2                                # [G*K*K=784, H*W]
  out[c,p] = sum_k weight[g(c)*49+k, p] * x[c, p+dk] + x[c, p]

Sharding: 8 cores = 2 batches x 4 row-slices of 16 image rows each.
Each core computes its full pipeline on its slice (halo rows come in
via host-side padding); no cross-core communication.

Engine plan (v2):
  - PE: mm1, then one fused "broadcast" matmul per tap with
    lhsT = w2e[65,128] (w2 rearranged per channel; row 64 carries the
    b2 bias via a ones-row appended to t; center tap bias += 1 folds
    in the +x residual). PSUM directly holds per-channel tap weights.
  - Scalar (ACT): BN+relu on t, PSUM->SBUF fp16 weight copies.
  - DVE: the involution multiplies and adds only, all contiguous fp16
    SBUF ops (2x mode). x arrives as 7 kw-shifted contiguous copies
    DMA'd straight from HBM (host ships the padded fp16 feature map).
  - GPSIMD: unused (concurrent GPSIMD SBUF reads starve the DVE).
"""

import numpy as np

import concourse.bass as bass
import concourse.bacc as bacc
import concourse.mybir as mybir
import concourse.tile as tile
from concourse.bass_utils import run_bass_kernel_spmd

FP = mybir.dt.float32
HP = mybir.dt.float16
N_CORES = 8
C = 256
RED = 64
G = 16
GC = 16
KS = 7
KK = KS * KS  # 49
H = W = 64
ROWS = 16          # image rows per core
PIX = ROWS * W     # 1024 pixels per core
PROWS = ROWS + 6   # padded rows (halo 3 each side)
PW = W + 6         # padded width
XROW = PROWS * W   # 1408: one kw-shifted copy, rows contiguous at 64

TRACE = False
LAST_RESULTS = None

_CACHED_NC = None


def _build_nc():
    nc = bacc.Bacc(
        "TRN2",
        debug=False,
        target_bir_lowering=False,
        num_devices=N_CORES,
    )

    guide = nc.dram_tensor("guide", (C, ROWS, W), HP, kind="ExternalInput")
    # feat7[c, kw, r, x] = padded_feature[c, r, x + kw] (host-shifted, so
    # every load and every DVE multiply is contiguous at 64-wide rows)
    feat7 = nc.dram_tensor("feat7", (C, KS, PROWS, W), HP,
                           kind="ExternalInput")
    w1t = nc.dram_tensor("w1t", (C, RED), HP, kind="ExternalInput")
    # w2a[r, ((i*7+kw)*7+kh)*128 + c] = w2[(c//16+8i)*49 + kh*7+kw, r];
    # row RED carries b2 (+1 on center tap -> +x residual)
    w2a = nc.dram_tensor("w2a", (RED + 1, 2 * KK * 128), HP,
                         kind="ExternalInput")
    scl = nc.dram_tensor("scl", (RED, 1), FP, kind="ExternalInput")
    shf = nc.dram_tensor("shf", (RED, 1), FP, kind="ExternalInput")
    out = nc.dram_tensor("out", (C, ROWS, W), HP, kind="ExternalOutput")

    MUL = mybir.AluOpType.mult
    ADD = mybir.AluOpType.add
    # taps per PSUM chunk: {2,2,2,1} per kw-group (PSUM = 8 banks =
    # 4096 fp32; 2-tap chunk = 2048 fp32 = 4 banks, ping-ponged)
    CHUNKS = [(0, 2), (2, 2), (4, 2), (6, 1)]

    with tile.TileContext(nc) as tc:
        with (
            tc.tile_pool(name="consts", bufs=1) as consts,
            tc.tile_pool(name="big", bufs=1) as big,
            tc.tile_pool(name="work", bufs=3) as work,
            tc.tile_pool(name="psum", bufs=2, space="PSUM") as psum,
        ):
            # ---- ACT table preload: tiny dep-free op at t=0 ----
            warm = consts.tile([1, 1], FP)
            nc.vector.memset(warm, 0.0)
            nc.scalar.activation(warm, warm,
                                 mybir.ActivationFunctionType.Relu)

            # ---- loads ----
            scl_sb = consts.tile([RED, 1], FP)
            nc.sync.dma_start(scl_sb, scl.ap())
            shf_sb = consts.tile([RED, 1], FP)
            nc.sync.dma_start(shf_sb, shf.ap())
            w1t_sb = [consts.tile([128, RED], HP, tag=f"w1t{i}",
                                  name=f"w1t{i}") for i in range(2)]
            guide_sb = [big.tile([128, PIX], HP, tag=f"gd{i}", name=f"gd{i}")
                        for i in range(2)]
            for i in range(2):
                csl = slice(i * 128, (i + 1) * 128)
                nc.sync.dma_start(
                    guide_sb[i],
                    guide.ap()[csl].rearrange("p a b -> p (a b)"))
                nc.sync.dma_start(w1t_sb[i], w1t.ap()[i * 128:(i + 1) * 128])
            w2a_sb = consts.tile([RED + 1, 2 * KK * 128], HP)
            # per-(half, kw) chunks of w2a and xk stream in just-in-time
            # (emitted inside the kw loop, after the first groups' chunks)
            xk = [big.tile([128, KS * XROW], HP, tag=f"xk{i}", name=f"xk{i}")
                  for i in range(2)]

            def load_group(i, kw):
                csl = slice(i * 128, (i + 1) * 128)
                nc.sync.dma_start(
                    w2a_sb[:, (i * KS + kw) * KS * 128:
                           (i * KS + kw + 1) * KS * 128],
                    w2a.ap()[:, (i * KS + kw) * KS * 128:
                             (i * KS + kw + 1) * KS * 128])
                nc.sync.dma_start(
                    xk[i][:, kw * XROW:(kw + 1) * XROW],
                    feat7.ap()[csl, kw].rearrange("p a b -> p (a b)"))

            # first two groups' weight/feature chunks go on the queue now
            load_group(0, 0)
            load_group(0, 1)

            # ---- mm1 + BN/relu -> t (fp16), plus ones row for bias ----
            t_sb = big.tile([RED + 1, PIX], HP)
            nc.vector.memset(t_sb[RED:RED + 1, :], 1.0)
            for nh in range(2):
                nsl = slice(nh * 512, (nh + 1) * 512)
                t_ps = psum.tile([RED, 512], FP, tag="wb",
                                 padded_shape=[RED, 2048])
                for i in range(2):
                    nc.tensor.matmul(
                        t_ps,
                        w1t_sb[i],
                        guide_sb[i][:, nsl],
                        start=(i == 0),
                        stop=(i == 1),
                    )
                nc.scalar.activation(
                    t_sb[0:RED, nsl], t_ps,
                    mybir.ActivationFunctionType.Relu,
                    bias=shf_sb[:, :], scale=scl_sb[:, :],
                )

            # ---- involution ----
            for i in range(2):
                run = big.tile([128, PIX], HP, tag=f"run{i}", name=f"run{i}")
                osb = big.tile([128, PIX], HP, tag=f"osb{i}", name=f"osb{i}")
                for kw in range(KS):
                    # prefetch the group two steps ahead
                    nxt = i * KS + kw + 2
                    if nxt < 2 * KS:
                        load_group(nxt // KS, nxt % KS)
                    # broadcast matmuls + fp16 copy for the 7 kh-taps
                    wsb = work.tile([128, KS * PIX], HP, tag="wsb", bufs=2)
                    prod = work.tile([128, KS * PIX], HP, tag="prod", bufs=2)
                    head = (i == 0 and kw == 0) or (i == 1 and kw == KS - 1)
                    for (k0, nk) in CHUNKS:
                        w_ps = psum.tile([128, nk * PIX], FP, tag="wb",
                                         padded_shape=[128, 2048])
                        for u in range(nk):
                            kh = k0 + u
                            lhs = w2a_sb[:, ((i * KS + kw) * KS + kh) * 128:
                                         ((i * KS + kw) * KS + kh + 1) * 128]
                            for nh in range(2):
                                nc.tensor.matmul(
                                    w_ps[:, u * PIX + nh * 512:
                                         u * PIX + nh * 512 + 512],
                                    lhs,
                                    t_sb[:, nh * 512:nh * 512 + 512],
                                    start=True,
                                    stop=True,
                                )
                        nc.scalar.activation(
                            wsb[:, k0 * PIX:(k0 + nk) * PIX], w_ps,
                            mybir.ActivationFunctionType.Copy,
                        )
                        if head:
                            # pipeline head: multiply per-chunk so the
                            # DVE starts before the whole group is copied
                            xs = bass.AP(
                                tensor=xk[i].tensor,
                                offset=xk[i].offset + kw * XROW + k0 * W,
                                ap=[xk[i].ap[0], [W, nk], [1, PIX]],
                            )
                            nc.vector.tensor_tensor(
                                prod[:, k0 * PIX:(k0 + nk) * PIX].rearrange(
                                    "p (a b) -> p a b", b=PIX),
                                xs,
                                wsb[:, k0 * PIX:(k0 + nk) * PIX].rearrange(
                                    "p (a b) -> p a b", b=PIX),
                                MUL)
                    if not head:
                        # multiply all 7 kh-taps: runs of 1024 contiguous
                        xs = bass.AP(
                            tensor=xk[i].tensor,
                            offset=xk[i].offset + kw * XROW,
                            ap=[xk[i].ap[0], [W, KS], [1, PIX]],
                        )
                        nc.vector.tensor_tensor(
                            prod.rearrange("p (a b) -> p a b", b=PIX),
                            xs,
                            wsb.rearrange("p (a b) -> p a b", b=PIX),
                            MUL)
                    # 7 -> 1 tree (contiguous fp16), accumulated into run
                    s2 = work.tile([128, 3 * PIX], HP, tag="s2", bufs=2)
                    nc.vector.tensor_tensor(
                        s2, prod[:, 0:3 * PIX], prod[:, 3 * PIX:6 * PIX], ADD)
                    ab = work.tile([128, 2 * PIX], HP, tag="ab", bufs=2)
                    nc.vector.tensor_tensor(
                        ab[:, 0:PIX], s2[:, 0:PIX], s2[:, PIX:2 * PIX], ADD)
                    nc.vector.tensor_tensor(
                        ab[:, PIX:2 * PIX], s2[:, 2 * PIX:3 * PIX],
                        prod[:, 6 * PIX:7 * PIX], ADD)
                    if kw == 0:
                        nc.vector.tensor_tensor(
                            run, ab[:, 0:PIX], ab[:, PIX:2 * PIX], ADD)
                    else:
                        g = work.tile([128, PIX], HP, tag="g", bufs=2)
                        nc.vector.tensor_tensor(
                            g, ab[:, 0:PIX], ab[:, PIX:2 * PIX], ADD)
                        dst = osb if kw == KS - 1 else run
                        nc.vector.tensor_tensor(dst, run, g, ADD)
                nc.sync.dma_start(
                    out.ap()[i * 128:(i + 1) * 128].rearrange(
                        "p a b -> p (a b)"),
                    osb)

    nc.compile()
    return nc


def kernel(**inputs):
    global _CACHED_NC, LAST_RESULTS
    feature_map = np.asarray(inputs["feature_map"], np.float32)
    guide_map = np.asarray(inputs["guide_map"], np.float32)
    w1 = np.asarray(inputs["w1"], np.float32)
    bn_gamma = np.asarray(inputs["bn_gamma"], np.float32)
    bn_beta = np.asarray(inputs["bn_beta"], np.float32)
    bn_mean = np.asarray(inputs["bn_mean"], np.float32)
    bn_var = np.asarray(inputs["bn_var"], np.float32)
    w2 = np.asarray(inputs["w2"], np.float32)
    b2 = np.asarray(inputs["b2"], np.float32)

    scale = bn_gamma / np.sqrt(bn_var + 1e-5)
    shift = bn_beta - bn_mean * scale
    w1t = np.ascontiguousarray(w1.T).astype(np.float16)    # [256, 64]

    # w2a[r, i, kw, kh, c] = w2[(c//16 + 8i)*49 + kh*7+kw, r]; row RED =
    # b2 bias (+1 on the center tap: folds the +x residual in)
    w2g = w2.reshape(G, KS, KS, RED)                        # [g, kh, kw, r]
    b2g = b2.reshape(G, KS, KS).copy()                      # [g, kh, kw]
    b2g[:, 3, 3] += 1.0
    w2a = np.zeros((RED + 1, 2, KS, KS, 128), np.float32)
    for i in range(2):
        gidx = np.arange(128) // GC + 8 * i                 # [c] -> g
        # [r, kw, kh, c]
        w2a[0:RED, i] = w2g[gidx].transpose(3, 2, 1, 0)
        w2a[RED, i] = b2g[gidx].transpose(2, 1, 0)
    w2a = np.ascontiguousarray(
        w2a.reshape(RED + 1, 2 * KK * 128)).astype(np.float16)

    fpad = np.pad(feature_map, ((0, 0), (0, 0), (3, 3), (3, 3))).astype(
        np.float16)
    # feat7[b, c, kw, r, x] = fpad[b, c, r, x + kw]
    feat7 = np.stack([fpad[:, :, :, kw:kw + W] for kw in range(KS)], axis=2)

    in_maps = []
    for core in range(N_CORES):
        b, q = divmod(core, 4)
        r0 = q * ROWS
        in_maps.append(dict(
            guide=np.ascontiguousarray(
                guide_map[b, :, r0:r0 + ROWS, :]).astype(np.float16),
            feat7=np.ascontiguousarray(feat7[b, :, :, r0:r0 + PROWS, :]),
            w1t=w1t, w2a=w2a,
            scl=scale.reshape(-1, 1), shf=shift.reshape(-1, 1),
        ))

    if _CACHED_NC is None:
        _CACHED_NC = _build_nc()
    nc = _CACHED_NC

    br = run_bass_kernel_spmd(
        nc, in_maps, list(range(N_CORES)), trace=TRACE,
    )
    LAST_RESULTS = br

    out = np.empty((2, C, H, W), np.float32)
    for core in range(N_CORES):
        b, q = divmod(core, 4)
        r0 = q * ROWS
        out[b, :, r0:r0 + ROWS, :] = br.results[core]["out"].astype(
            np.float32)
    return out
